# revision 4
# baseline (speedup 1.0000x reference)
"""NT-Xent loss kernel for Trainium2, 8 NeuronCores.

Row-sharded similarity matrix, multi-engine exponential pipeline:
  - Each core gets the full feature matrix cyclically rolled by c*1024 rows
    (identical SPMD program; its rows are always rolled-rows [0,1024)).
  - Preamble (per 2048-col group): DMA-in x f32; row norms via
    square(GPSIMD)+reduce(DVE); rnorm = exp(-0.5 ln) on ACT; scale to
    zb = x * 32/||x|| in bf16 (broadcast tensor_tensor); transpose via the
    SBUF crossbar DMA (dma_start_transpose, free - no PE/DVE time); cast to
    fp8e4 (GPSIMD); partition-fold into the DoubleRow layout
    z8dr[64, 2, cols] with two SBUF-SBUF DMAs.
  - Main loop (32 units of [128 rows x 2048 cols]): fp8 DoubleRow matmuls
    (2 moving cols/cycle - PE time halved vs bf16) into PSUM f32; the exp
    work is split between ACT (native Exp + fused accum_out row sums, cols
    [0:ACOLS]) and DVE (Schraudolph exp: one fused tensor_scalar
    f32->int16 whose codes ARE the bf16 bit pattern of exp(x), plus a 2x
    bf16 tensor_reduce for row sums, cols [ACOLS:2048]).
  - The diagonal/positive-pair blocks (units g=0 and g=2, cols m*128..)
    land in the ACT range; those [128,128] code blocks are DMA'd out raw
    and the host extracts diagonals, does ln(rowsum - exp_diag) -
    ln(exp_pos), and means - no device epilogue.
"""

import os

import numpy as np

N = 8192
D = 128
NCORES = 8
RPC = N // NCORES          # rows per core = 1024
G = 4                      # column groups
GCOLS = N // G             # 2048 columns per group
M = RPC // 128             # row tiles per core = 8
ACOLS = 1344               # ACT's exp share per 2048-col tile (mult of 128)
ZSCALE = 32.0              # z is scaled by 32 before fp8e4 quantization
ESC = 10.0 / (ZSCALE * ZSCALE)   # exp scale on raw psum values
SCH_A = 1.8033688011       # ESC * 2^7 * log2(e)   (Schraudolph slope)
SCH_B = 16248.7807255      # 127*2^7 mean-centered (Schraudolph offset)

_CACHE = {}
LAST_RESULTS = None


def _patch_act_tables():
    """Force Exp/Ln onto the combined natural_log_exp_and_others table set
    so a single ACT table load covers the whole kernel."""
    if _CACHE.get("act_patched"):
        return
    import functools

    import concourse.bacc as bacc_mod
    import concourse.bass_interp as interp_mod
    import concourse.hw_specs as hw_specs
    import concourse.mybir as mybir

    AF = mybir.ActivationFunctionType
    orig = hw_specs.get_activation_tables

    @functools.cache
    def patched(arch):
        out = {}
        for name, funcs in orig(arch).items():
            if name != "natural_log_exp_and_others":
                funcs = funcs - {AF.Exp, AF.Ln}
            out[name] = funcs
        return out

    hw_specs.get_activation_tables = patched
    bacc_mod.get_activation_tables = patched
    interp_mod.get_activation_tables = patched
    _CACHE["act_patched"] = True


def _patch_ldw_opt():
    """Let walrus dedup consecutive identical LDWEIGHTS; the DoubleRow
    stationary is reused by 4 consecutive matmuls per unit."""
    if _CACHE.get("ldw_patched"):
        return
    import concourse.bass_utils as bu

    orig = bu.run_command

    def run2(argv, **kw):
        argv = [
            "--enable-ldw-opt=true" if a == "--enable-ldw-opt=false" else a
            for a in argv
        ]
        return orig(argv, **kw)

    bu.run_command = run2
    _CACHE["ldw_patched"] = True


def _build():
    import concourse.mybir as mybir
    import concourse.tile as tile
    from concourse import bacc

    _patch_act_tables()  # (ldw-opt is incompatible with DoubleRow ldweights)

    f32 = mybir.dt.float32
    bf16 = mybir.dt.bfloat16
    i16 = mybir.dt.int16
    fp8 = mybir.dt.float8e4
    AX = mybir.AxisListType
    OP = mybir.AluOpType
    AF = mybir.ActivationFunctionType
    DR = mybir.MatmulPerfMode.DoubleRow

    nc = bacc.Bacc(
        "TRN2",
        target_bir_lowering=False,
        debug=False,
        enable_asserts=False,
        num_devices=NCORES,
    )
    x = nc.dram_tensor("x", [N, D], f32, kind="ExternalInput").ap()
    racc_out = nc.dram_tensor("racc", [128, 2 * G * M], f32, kind="ExternalOutput").ap()
    etblk_out = nc.dram_tensor("etblk", [128, 2 * RPC], i16, kind="ExternalOutput").ap()

    with tile.TileContext(nc) as tc:
        with (
            tc.tile_pool(name="const", bufs=1) as constp,
            tc.tile_pool(name="xin", bufs=2) as xinp,
            tc.tile_pool(name="sq", bufs=2) as sqp,
            tc.tile_pool(name="small", bufs=2) as smallp,
            tc.tile_pool(name="zb", bufs=2) as zbp,
            tc.tile_pool(name="zbT", bufs=2) as zbTp,
            tc.tile_pool(name="z8T", bufs=2) as z8Tp,
            tc.tile_pool(name="z8dr", bufs=2) as z8drp,
            tc.tile_pool(name="z8dr0", bufs=1) as z8dr0p,
            tc.tile_pool(name="et", bufs=3) as etp,
            tc.tile_pool(name="acc", bufs=1) as accp,
            tc.tile_pool(name="psum", bufs=2, space="PSUM") as psump,
        ):
            # constants
            eps2 = constp.tile([128, 1], f32, tag="eps2")
            nc.vector.memset(eps2[:], 1e-16)
            ln32 = constp.tile([128, 1], f32, tag="ln32")
            nc.vector.memset(ln32[:], float(np.log(ZSCALE)))

            # warm the ACT Ln/Exp table during input DMA
            warm = constp.tile([128, 1], f32, tag="warm")
            nc.vector.memset(warm[:], 1.0)
            nc.scalar.activation(warm[:], warm[:], AF.Ln)
            nc.scalar.activation(warm[:], warm[:], AF.Exp)

            racc = accp.tile([128, 2 * G * M], f32, tag="racc")

            xgs = [None] * G
            z8dr = [None] * G

            def load_part(g, eng=None):
                """DMA group g's 2048 rolled rows of x into SBUF."""
                xg = xinp.tile([128, GCOLS], f32, tag="xg")
                for q in range(4):
                    src = x[g * GCOLS + q * 512 : g * GCOLS + (q + 1) * 512, :]
                    src = src.rearrange("(s p) d -> p s d", p=128)
                    dst = xg[:, q * 512 : (q + 1) * 512].rearrange(
                        "p (s d) -> p s d", s=4
                    )
                    e = eng if eng is not None else nc.sync
                    e.dma_start(out=dst, in_=src)
                xgs[g] = xg

            def prep_part(g, head):
                """Norms, scale, transpose, fp8, DoubleRow fold for group g.

                head=True (group 0) runs the elementwise steps on DVE (2x
                modes, shortest latency chain); otherwise on GPSIMD so the
                DVE keeps its exp budget.
                """
                xg = xgs[g]
                ew = nc.vector if head else nc.gpsimd

                # ||x||^2 per row (squares in bf16, free-axis reduce on DVE)
                sq = sqp.tile([128, GCOLS], bf16, tag="sq")
                ew.tensor_tensor(sq[:], xg[:], xg[:], op=OP.mult)
                nsq = smallp.tile([128, 16], bf16, tag="nsq")
                with nc.allow_low_precision(reason="bf16 norm accum, 2e-2 tol"):
                    nc.vector.tensor_reduce(
                        nsq[:],
                        sq[:].rearrange("p (s d) -> p s d", s=16),
                        axis=AX.X,
                        op=OP.add,
                    )
                # rno = 32 / ||x||  (exp(-0.5 ln(nsq + eps^2) + ln 32))
                lnv = smallp.tile([128, 16], f32, tag="lnv")
                nc.scalar.activation(lnv[:], nsq[:], AF.Ln, bias=eps2[:, 0:1])
                rno = smallp.tile([128, 16], f32, tag="rno")
                nc.scalar.activation(
                    rno[:], lnv[:], AF.Exp, scale=-0.5, bias=ln32[:, 0:1]
                )
                # zb = x * rno (bf16), broadcast along d
                zb = zbp.tile([128, GCOLS], bf16, tag="zb")
                ew.tensor_tensor(
                    zb[:].rearrange("p (s d) -> p s d", s=16),
                    xg[:].rearrange("p (s d) -> p s d", s=16),
                    rno[:].unsqueeze(-1).broadcast_to([128, 16, 128]),
                    op=OP.mult,
                )
                # crossbar transpose: zbT[d, s*128+r] = zb[r, s*128+d]
                zbT = zbTp.tile([128, GCOLS], bf16, tag="zbT")
                nc.sync.dma_start_transpose(
                    out=zbT[:].rearrange("p (s r) -> p s r", s=16),
                    in_=zb[:],
                )
                # fp8 quantization
                z8T = z8Tp.tile([128, GCOLS], fp8, tag="z8T")
                ew.tensor_copy(z8T[:], zbT[:])
                # DoubleRow partition fold: z8dr[p, i, c] = z8T[64i + p, c]
                pool = z8dr0p if g == 0 else z8drp
                zd = pool.tile([64, 2, GCOLS], fp8, tag="z8dr")
                for i in range(2):
                    nc.sync.dma_start(
                        out=zd[:, i, :], in_=z8T[64 * i : 64 * i + 64, :]
                    )
                z8dr[g] = zd

            def unit(g, m):
                """One [128 rows x 2048 cols] tile: matmuls + split exp."""
                u = g * M + m
                pt = psump.tile([128, GCOLS], f32, tag="pt")
                lhs = z8dr[0][:, :, m * 128 : (m + 1) * 128]
                for k in range(4):
                    nc.tensor.matmul(
                        pt[:, k * 512 : (k + 1) * 512],
                        lhs,
                        z8dr[g][:, :, k * 512 : (k + 1) * 512],
                        perf_mode=DR,
                    )
                et = etp.tile([128, GCOLS], i16, tag="et")
                # ACT: true exp, fused row-sum accumulation
                nc.scalar.activation(
                    et[:, 0:ACOLS].bitcast(bf16),
                    pt[:, 0:ACOLS],
                    AF.Exp,
                    scale=ESC,
                    accum_out=racc[:, u : u + 1],
                )
                # DVE: Schraudolph exp (int16 codes = bf16 bits), 2x reduce
                nc.vector.tensor_scalar(
                    et[:, ACOLS:GCOLS], pt[:, ACOLS:GCOLS],
                    SCH_A, SCH_B, OP.mult, OP.add,
                )
                nc.vector.tensor_reduce(
                    racc[:, G * M + u : G * M + u + 1],
                    et[:, ACOLS:GCOLS].bitcast(bf16),
                    axis=AX.X,
                    op=OP.add,
                )
                # diag (g=0) / positive (g=2) code blocks out for the host
                if g == 0 or g == 2:
                    half = 0 if g == 0 else 1
                    nc.sync.dma_start(
                        out=etblk_out[:, half * RPC + m * 128 : half * RPC + (m + 1) * 128],
                        in_=et[:, m * 128 : (m + 1) * 128],
                    )

            # ---- pipeline ----
            load_part(0)
            prep_part(0, head=True)
            for g in range(G):
                for m in range(M):
                    unit(g, m)
                    if g + 1 < G:
                        if m == 0:
                            load_part(g + 1)
                        elif m == 3:
                            prep_part(g + 1, head=False)

            nc.sync.dma_start(out=racc_out, in_=racc[:])

    nc.compile()
    return nc


def _get_nc():
    if "nc" not in _CACHE:
        _CACHE["nc"] = _build()
    return _CACHE["nc"]


def kernel(stacked_batch: np.ndarray) -> np.ndarray:
    global LAST_RESULTS
    import ml_dtypes
    from concourse.bass_utils import run_bass_kernel_spmd

    nc = _get_nc()
    xf = np.ascontiguousarray(np.asarray(stacked_batch, dtype=np.float32))
    assert xf.shape == (N, D)

    in_maps = [
        {"x": np.ascontiguousarray(np.roll(xf, -c * RPC, axis=0))}
        for c in range(NCORES)
    ]
    res = run_bass_kernel_spmd(
        nc,
        in_maps,
        core_ids=list(range(NCORES)),
        trace=bool(os.environ.get("BASS_TRACE")),
    )
    LAST_RESULTS = res

    total = 0.0
    idx = np.arange(128)
    for c in range(NCORES):
        racc = np.asarray(res.results[c]["racc"], dtype=np.float64)  # [128, 64]
        etblk = np.asarray(res.results[c]["etblk"])                  # [128, 2048] i16
        vals = etblk.view(np.uint16).view(ml_dtypes.bfloat16).astype(np.float64)
        # rowsum for local row r = m*128 + p: sum ACT + DVE partials over g
        rowsum = np.zeros((128, M))
        for g in range(G):
            for m in range(M):
                u = g * M + m
                rowsum[:, m] += racc[:, u] + racc[:, G * M + u]
        exp_diag = np.empty((128, M))
        exp_pos = np.empty((128, M))
        for m in range(M):
            exp_diag[:, m] = vals[idx, m * 128 + idx]
            exp_pos[:, m] = vals[idx, RPC + m * 128 + idx]
        loss = np.log(rowsum - exp_diag) - np.log(exp_pos)
        total += float(loss.sum())
    return np.float32(total / N)


# revision 6
# speedup vs baseline: 1.2203x; 1.2203x over previous
"""NT-Xent loss kernel for Trainium2, 8 NeuronCores.

Row-sharded similarity matrix, multi-engine exponential pipeline:
  - Each core gets the full feature matrix cyclically rolled by c*1024 rows
    (identical SPMD program; its rows are always rolled-rows [0,1024)).
  - Preamble (per 2048-col group): DMA-in x f32; row norms via
    square+reduce; rnorm = exp(-0.5 ln) on ACT; scale to z in bf16
    (broadcast tensor_tensor); transpose via the SBUF crossbar DMA
    (dma_start_transpose - no PE/DVE time).  Group 0's elementwise work
    runs on DVE (short head chain); later groups' on GPSIMD.
  - Main loop (32 units of [128 rows x 2048 cols]): bf16 matmuls (fp8
    DoubleRow measured SLOWER than bf16 on hw) into PSUM f32; the exp
    work is split between ACT (native Exp + fused accum_out row sums, cols
    [0:ACOLS]) and DVE (Schraudolph exp: one fused tensor_scalar
    f32->int16 whose codes ARE the bf16 bit pattern of exp(x), plus a 2x
    bf16 tensor_reduce for row sums, cols [ACOLS:2048]).
  - The diagonal/positive-pair blocks (units g=0 and g=2, cols m*128..)
    land in the ACT range; those [128,128] code blocks are DMA'd out raw
    and the host extracts diagonals, does ln(rowsum - exp_diag) -
    ln(exp_pos), and means - no device epilogue.
"""

import os

import numpy as np

N = 8192
D = 128
NCORES = 8
RPC = N // NCORES          # rows per core = 1024
G = 4                      # column groups
GCOLS = N // G             # 2048 columns per group
M = RPC // 128             # row tiles per core = 8
ACOLS = 1664               # ACT's exp share per 2048-col tile (mult of 128)
ZSCALE = 1.0               # z kept at unit scale (bf16 matmul)
ESC = 10.0                 # exp scale on raw psum values
SCH_A = 1846.649652        # ESC * 2^7 * log2(e)   (Schraudolph slope)
SCH_B = 16248.7807255      # 127*2^7 mean-centered (Schraudolph offset)

_CACHE = {}
LAST_RESULTS = None


def _patch_act_tables():
    """Force Exp/Ln onto the combined natural_log_exp_and_others table set
    so a single ACT table load covers the whole kernel."""
    if _CACHE.get("act_patched"):
        return
    import functools

    import concourse.bacc as bacc_mod
    import concourse.bass_interp as interp_mod
    import concourse.hw_specs as hw_specs
    import concourse.mybir as mybir

    AF = mybir.ActivationFunctionType
    orig = hw_specs.get_activation_tables

    @functools.cache
    def patched(arch):
        out = {}
        for name, funcs in orig(arch).items():
            if name != "natural_log_exp_and_others":
                funcs = funcs - {AF.Exp, AF.Ln}
            out[name] = funcs
        return out

    hw_specs.get_activation_tables = patched
    bacc_mod.get_activation_tables = patched
    interp_mod.get_activation_tables = patched
    _CACHE["act_patched"] = True


def _patch_ldw_opt():
    """Let walrus dedup consecutive identical LDWEIGHTS; the DoubleRow
    stationary is reused by 4 consecutive matmuls per unit."""
    if _CACHE.get("ldw_patched"):
        return
    import concourse.bass_utils as bu

    orig = bu.run_command

    def run2(argv, **kw):
        argv = [
            "--enable-ldw-opt=true" if a == "--enable-ldw-opt=false" else a
            for a in argv
        ]
        return orig(argv, **kw)

    bu.run_command = run2
    _CACHE["ldw_patched"] = True


def _build():
    import concourse.mybir as mybir
    import concourse.tile as tile
    from concourse import bacc

    _patch_act_tables()

    f32 = mybir.dt.float32
    bf16 = mybir.dt.bfloat16
    i16 = mybir.dt.int16
    AX = mybir.AxisListType
    OP = mybir.AluOpType
    AF = mybir.ActivationFunctionType

    nc = bacc.Bacc(
        "TRN2",
        target_bir_lowering=False,
        debug=False,
        enable_asserts=False,
        num_devices=NCORES,
    )
    x = nc.dram_tensor("x", [N, D], f32, kind="ExternalInput").ap()
    racc_out = nc.dram_tensor("racc", [128, 2 * G * M], f32, kind="ExternalOutput").ap()
    etblk_out = nc.dram_tensor("etblk", [128, 2 * RPC], i16, kind="ExternalOutput").ap()

    with tile.TileContext(nc) as tc:
        with (
            tc.tile_pool(name="const", bufs=1) as constp,
            tc.tile_pool(name="xin", bufs=2) as xinp,
            tc.tile_pool(name="sq", bufs=2) as sqp,
            tc.tile_pool(name="small", bufs=2) as smallp,
            tc.tile_pool(name="zb", bufs=2) as zbp,
            tc.tile_pool(name="zbT", bufs=2) as zbTp,
            tc.tile_pool(name="zbT0", bufs=1) as zbT0p,
            tc.tile_pool(name="et", bufs=3) as etp,
            tc.tile_pool(name="acc", bufs=1) as accp,
            tc.tile_pool(name="psum", bufs=2, space="PSUM") as psump,
        ):
            # constants
            eps2 = constp.tile([128, 1], f32, tag="eps2")
            nc.vector.memset(eps2[:], 1e-16)
            ln32 = constp.tile([128, 1], f32, tag="ln32")
            nc.vector.memset(ln32[:], float(np.log(ZSCALE)))

            # warm the ACT Ln/Exp table during input DMA
            warm = constp.tile([128, 1], f32, tag="warm")
            nc.vector.memset(warm[:], 1.0)
            nc.scalar.activation(warm[:], warm[:], AF.Ln)
            nc.scalar.activation(warm[:], warm[:], AF.Exp)

            racc = accp.tile([128, 2 * G * M], f32, tag="racc")

            xgs = [None] * G
            zbTs = [None] * G

            def load_part(g, eng=None):
                """DMA group g's 2048 rolled rows of x into SBUF."""
                xg = xinp.tile([128, GCOLS], f32, tag="xg")
                for q in range(4):
                    src = x[g * GCOLS + q * 512 : g * GCOLS + (q + 1) * 512, :]
                    src = src.rearrange("(s p) d -> p s d", p=128)
                    dst = xg[:, q * 512 : (q + 1) * 512].rearrange(
                        "p (s d) -> p s d", s=4
                    )
                    e = eng if eng is not None else nc.sync
                    e.dma_start(out=dst, in_=src)
                xgs[g] = xg

            def prep_part(g, head):
                """Norms, scale, transpose, fp8, DoubleRow fold for group g.

                head=True (group 0) runs the elementwise steps on DVE (2x
                modes, shortest latency chain); otherwise on GPSIMD so the
                DVE keeps its exp budget.
                """
                xg = xgs[g]
                ew = nc.vector if head else nc.gpsimd

                # ||x||^2 per row (squares in bf16, free-axis reduce on DVE)
                sq = sqp.tile([128, GCOLS], bf16, tag="sq")
                ew.tensor_tensor(sq[:], xg[:], xg[:], op=OP.mult)
                nsq = smallp.tile([128, 16], bf16, tag="nsq")
                with nc.allow_low_precision(reason="bf16 norm accum, 2e-2 tol"):
                    nc.vector.tensor_reduce(
                        nsq[:],
                        sq[:].rearrange("p (s d) -> p s d", s=16),
                        axis=AX.X,
                        op=OP.add,
                    )
                # rno = 32 / ||x||  (exp(-0.5 ln(nsq + eps^2) + ln 32))
                lnv = smallp.tile([128, 16], f32, tag="lnv")
                nc.scalar.activation(lnv[:], nsq[:], AF.Ln, bias=eps2[:, 0:1])
                rno = smallp.tile([128, 16], f32, tag="rno")
                nc.scalar.activation(
                    rno[:], lnv[:], AF.Exp, scale=-0.5, bias=ln32[:, 0:1]
                )
                # zb = x * rno (bf16), broadcast along d
                zb = zbp.tile([128, GCOLS], bf16, tag="zb")
                ew.tensor_tensor(
                    zb[:].rearrange("p (s d) -> p s d", s=16),
                    xg[:].rearrange("p (s d) -> p s d", s=16),
                    rno[:].unsqueeze(-1).broadcast_to([128, 16, 128]),
                    op=OP.mult,
                )
                # crossbar transpose: zbT[d, s*128+r] = zb[r, s*128+d]
                pool = zbT0p if g == 0 else zbTp
                zbT = pool.tile([128, GCOLS], bf16, tag="zbT")
                nc.sync.dma_start_transpose(
                    out=zbT[:].rearrange("p (s r) -> p s r", s=16),
                    in_=zb[:],
                )
                zbTs[g] = zbT

            def unit(g, m):
                """One [128 rows x 2048 cols] tile: matmuls + split exp."""
                u = g * M + m
                pt = psump.tile([128, GCOLS], f32, tag="pt")
                lhs = zbTs[0][:, m * 128 : (m + 1) * 128]
                for k in range(4):
                    nc.tensor.matmul(
                        pt[:, k * 512 : (k + 1) * 512],
                        lhs,
                        zbTs[g][:, k * 512 : (k + 1) * 512],
                    )
                et = etp.tile([128, GCOLS], i16, tag="et")
                # ACT: true exp, fused row-sum accumulation
                nc.scalar.activation(
                    et[:, 0:ACOLS].bitcast(bf16),
                    pt[:, 0:ACOLS],
                    AF.Exp,
                    scale=ESC,
                    accum_out=racc[:, u : u + 1],
                )
                # DVE: Schraudolph exp (int16 codes = bf16 bits), 2x reduce
                nc.vector.tensor_scalar(
                    et[:, ACOLS:GCOLS], pt[:, ACOLS:GCOLS],
                    SCH_A, SCH_B, OP.mult, OP.add,
                )
                nc.vector.tensor_reduce(
                    racc[:, G * M + u : G * M + u + 1],
                    et[:, ACOLS:GCOLS].bitcast(bf16),
                    axis=AX.X,
                    op=OP.add,
                )
                # diag (g=0) / positive (g=2) code blocks out for the host
                if g == 0 or g == 2:
                    half = 0 if g == 0 else 1
                    nc.sync.dma_start(
                        out=etblk_out[:, half * RPC + m * 128 : half * RPC + (m + 1) * 128],
                        in_=et[:, m * 128 : (m + 1) * 128],
                    )

            # ---- pipeline ----
            load_part(0)
            prep_part(0, head=True)
            for g in range(G):
                for m in range(M):
                    unit(g, m)
                    if g + 1 < G:
                        if m == 0:
                            load_part(g + 1)
                        elif m == 3:
                            prep_part(g + 1, head=False)

            nc.sync.dma_start(out=racc_out, in_=racc[:])

    nc.compile()
    return nc


def _get_nc():
    if "nc" not in _CACHE:
        _CACHE["nc"] = _build()
    return _CACHE["nc"]


def kernel(stacked_batch: np.ndarray) -> np.ndarray:
    global LAST_RESULTS
    import ml_dtypes
    from concourse.bass_utils import run_bass_kernel_spmd

    nc = _get_nc()
    xf = np.ascontiguousarray(np.asarray(stacked_batch, dtype=np.float32))
    assert xf.shape == (N, D)

    in_maps = [
        {"x": np.ascontiguousarray(np.roll(xf, -c * RPC, axis=0))}
        for c in range(NCORES)
    ]
    res = run_bass_kernel_spmd(
        nc,
        in_maps,
        core_ids=list(range(NCORES)),
        trace=bool(os.environ.get("BASS_TRACE")),
    )
    LAST_RESULTS = res

    total = 0.0
    idx = np.arange(128)
    for c in range(NCORES):
        racc = np.asarray(res.results[c]["racc"], dtype=np.float64)  # [128, 64]
        etblk = np.asarray(res.results[c]["etblk"])                  # [128, 2048] i16
        vals = etblk.view(np.uint16).view(ml_dtypes.bfloat16).astype(np.float64)
        # rowsum for local row r = m*128 + p: sum ACT + DVE partials over g
        rowsum = np.zeros((128, M))
        for g in range(G):
            for m in range(M):
                u = g * M + m
                rowsum[:, m] += racc[:, u] + racc[:, G * M + u]
        exp_diag = np.empty((128, M))
        exp_pos = np.empty((128, M))
        for m in range(M):
            exp_diag[:, m] = vals[idx, m * 128 + idx]
            exp_pos[:, m] = vals[idx, RPC + m * 128 + idx]
        loss = np.log(rowsum - exp_diag) - np.log(exp_pos)
        total += float(loss.sum())
    return np.float32(total / N)
